# revision 17
# baseline (speedup 1.0000x reference)
"""Chamfer 2D loss kernel for Trainium2 (8 NeuronCores, SPMD).

Problem: N=16 objects, two point sets [16, 4096, 2] fp32 each.
Per object: C[i,j] = sqrt(clip(|x_i|^2 + |y_j|^2 - 2 x_i.y_j, 1e-12));
loss = mean_n mask_n * 0.5 * (mean_i min_j C + mean_j min_i C).

Sharding: data-parallel over objects, 2 objects per core; per core 4
"generations" (2 objects x 2 directions), each computing the 4096
per-query min squared distances for query set Q vs key set K.

Per generation, per i-tile of 128 queries, ALL 4096 keys go through the
PE as a K=10 fp16 matmul that computes -d^2/2 = q.k - |k|^2/2 - |q|^2/2
directly in PSUM.  Full fp32 inputs are split hi/lo into fp16 pairs
(k = kh + kl), keeping ~17 mantissa bits through the product terms:
  r0 (1,    mh_k) r1 (1,    ml_k)      m = -|k|^2/2 split
  r2 (q0h, k0h) r3 (q0l, k0h) r4 (q0h, k0l)
  r5 (q1h, k1h) r6 (q1l, k1h) r7 (q1h, k1l)
  r8 (sqh_q, 1) r9 (sql_q, 1)          sq = -|q|^2/2 split
fp16 matmuls stream 1 column/cycle (vs 4 for fp32), so the PE is far off
the critical path.  Four 1024-key PSUM chunks per i-tile (2 banks each,
4-buf pool = all 8 banks): chunks N1,N2 are drained PSUM->SBUF by
ScalarE; chunks D1,D2 are read directly from PSUM by the custom DVE
max-fold (port0=PSUM, port1=SBUF, 2 keys/cycle), accumulating row maxima
of -d^2/2 into per-i-tile columns.  min_j d^2 = -2 * max.
min over j of sqrt == sqrt of min over j (monotonic), so only the 4096
per-query minima need sqrt, done on host with the means and the mask.
"""

import contextlib

import numpy as np

import concourse.bacc as bacc
import concourse.bass as bass
import concourse.tile as tile
from concourse import mybir
import concourse.dve_ops as dve_ops
from concourse.dve_ops import DveOp
from concourse.dve_spec import (
    Spec, Src0, Src1, C0, maxx, lower, _has_src1,
)
from concourse.dve_uop import DveOpSpec
from concourse.bass_utils import run_bass_kernel_spmd

F32 = mybir.dt.float32
F16 = mybir.dt.float16
ALU = mybir.AluOpType

N_CORES = 8
N_OBJ = 16
P = 4096          # points per set
OBJ_PER_CORE = N_OBJ // N_CORES      # 2
N_GENS = 2 * OBJ_PER_CORE            # 4 generations per core
IT = P // 128                        # 32 i-tiles of 128 queries
CH = 1024                            # keys per PSUM chunk (2 banks)
KR = 10                              # matmul contraction rows per group
EPS = 1e-12
NEG_BIG = -3.0e38


def _maxmax_ref(in0, in1, s0, s1, imm2):
    b = np.maximum(in0, in1).astype(np.float32)
    acc = np.maximum(
        np.asarray(s0, np.float32),
        b.reshape(b.shape[0], -1).max(axis=-1, keepdims=True),
    ).astype(np.float32)
    return b, acc


_REGISTERED = {}


def _register_op(name: str, spec: Spec) -> DveOp:
    """Register a custom DVE op at runtime. Tables are generated per-NEFF,
    so this needs no firmware support."""
    if name in _REGISTERED:
        return _REGISTERED[name]
    for op in dve_ops.OPS:
        if op.name == name:
            _REGISTERED[name] = op
            return op
    row = max(dve_ops._SUB_OPCODE_FOR_NAME.values()) + 1
    assert row < 0x20, "no free custom-DVE opcode row"
    dve_ops._SUB_OPCODE_FOR_NAME[name] = row
    shas = {
        ver: DveOpSpec(
            name=name, opcode=row, uops=lower(spec, ver=ver),
            rd1_en=_has_src1(spec),
        ).sha(ver)
        for ver in ("v3", "v4")
    }
    op = DveOp(name, spec, subdim=False, uops_sha=shas)
    dve_ops.OPS.append(op)
    dve_ops.CUSTOM_DVE_SPECS[name] = spec
    _REGISTERED[name] = op
    return op


def _get_ops():
    maxmax = _register_op(
        "TT_MAX_MAX_REDUCE_ANT",
        Spec(body=maxx(Src0, Src1), accum=maxx, accum_init=C0,
             reference=_maxmax_ref),
    )
    return maxmax


def _build_program(repeat: int = 1):
    """Build + compile the per-core Bass program. `repeat` re-runs the main
    compute loop inside a hardware For_i for timing (results unchanged)."""
    maxmax = _get_ops()

    nc = bacc.Bacc("TRN2", target_bir_lowering=False, debug=False)
    pts1 = nc.dram_tensor("pts1", [OBJ_PER_CORE, P, 2], F32, kind="ExternalInput")
    pts2 = nc.dram_tensor("pts2", [OBJ_PER_CORE, P, 2], F32, kind="ExternalInput")
    out = nc.dram_tensor("minsq", [N_GENS, 128, IT], F32, kind="ExternalOutput")

    ones_row = nc.inline_tensor(np.ones((1, P), dtype=np.float32), name="ones_row")
    scr = [nc.dram_tensor(f"scr{s}", [1, P], F32, kind="Internal") for s in range(4)]

    p1 = pts1.ap()
    p2 = pts2.ap()
    o = out.ap()

    # point sets in per-core order: s = 2*obj + (0: set1, 1: set2)
    set_aps = [p1[0], p2[0], p1[1], p2[1]]
    # generations: (query set idx, key set idx)
    gen_sets = [(0, 1), (1, 0), (2, 3), (3, 2)]

    with tile.TileContext(nc) as tc:
        with contextlib.ExitStack() as ctx:
            persist = ctx.enter_context(tc.tile_pool(name="persist", bufs=1))
            temps = ctx.enter_context(tc.tile_pool(name="temps", bufs=2))

            # discarded elementwise output of the custom DVE fold ops
            trash = persist.tile([128, CH], F32, tag="trash")
            qrows, krows = [], []
            with tc.tile_pool(name="prep_psum", bufs=2, space="PSUM") as ppsum, \
                 tc.tile_pool(name="prep", bufs=1) as prep:
                from concourse.masks import make_identity
                ident = prep.tile([128, 128], F32, tag="ident")
                make_identity(nc, ident[:])
                ones_bf = prep.tile([1, P], F16, tag="ones_bf")
                ones_sb = prep.tile([1, P], F32, tag="ones_sb")
                nc.sync.dma_start(out=ones_sb[:], in_=ones_row.ap()[:])
                nc.vector.tensor_copy(out=ones_bf[:], in_=ones_sb[:])

                def hi_lo(src_ap, n_part, tag):
                    """Split fp32 rows [n, P] into fp16 hi + lo tiles."""
                    hi = prep.tile([n_part, P], F16, tag=f"{tag}_hi", name="hi")
                    lo = prep.tile([n_part, P], F16, tag=f"{tag}_lo", name="lo")
                    hi32 = prep.tile([n_part, P], F32, tag="hl32", name="hi32")
                    nc.vector.tensor_copy(out=hi[:], in_=src_ap)
                    nc.vector.tensor_copy(out=hi32[:], in_=hi[:])
                    nc.vector.tensor_tensor(hi32[:], src_ap, hi32[:],
                                            op=ALU.subtract)
                    nc.vector.tensor_copy(out=lo[:], in_=hi32[:])
                    return hi, lo

                for s in range(4):
                    pset = set_aps[s]          # [P, 2] dram AP
                    rows2 = pset.rearrange("p d -> d p")              # [2, P]
                    cols = pset.rearrange("(c p) d -> p c d", p=128)  # [128, IT, 2]

                    # coordinate rows (fp32) -> fp16 hi/lo [2, P]
                    crows = prep.tile([2, P], F32, tag="crows")
                    nc.sync.dma_start(out=crows[:], in_=rows2)
                    chi, clo = hi_lo(crows[:], 2, "c")

                    # -|p|^2/2 row: build via column-wise square/sum, PE
                    # transpose, dram bounce -> [1, P] fp32 -> fp16 hi/lo
                    c0 = prep.tile([128, IT], F32, tag="qc0")
                    c1 = prep.tile([128, IT], F32, tag="qc1")
                    nc.sync.dma_start(out=c0[:], in_=cols[:, :, 0])
                    nc.sync.dma_start(out=c1[:], in_=cols[:, :, 1])
                    m0 = prep.tile([128, IT], F32, tag="m0")
                    sqc = prep.tile([128, IT], F32, tag="sqc")
                    nc.vector.tensor_tensor(m0[:], c0[:], c0[:], op=ALU.mult)
                    nc.vector.tensor_tensor(sqc[:], c1[:], c1[:], op=ALU.mult)
                    nc.vector.tensor_tensor(sqc[:], sqc[:], m0[:], op=ALU.add)
                    nsq = prep.tile([128, IT], F32, tag="nsq")
                    nc.vector.tensor_scalar_mul(nsq[:], sqc[:], -0.5)
                    pT = ppsum.tile([IT, 128], F32, tag="pT")
                    nc.tensor.transpose(pT[:], nsq[:], ident[:])
                    pTs = prep.tile([IT, 128], F32, tag="pTs")
                    nc.vector.tensor_copy(out=pTs[:], in_=pT[:])
                    nc.sync.dma_start(
                        out=scr[s].ap()[0].rearrange("(c p) -> c p", p=128),
                        in_=pTs[:],
                    )
                    srow = prep.tile([1, P], F32, tag="srow")
                    nc.sync.dma_start(out=srow[:], in_=scr[s].ap()[:])
                    shi, slo = hi_lo(srow[:], 1, "s")

                    # assemble the replicated row-group tiles (see module
                    # docstring for the 10-row pairing)
                    qg = [ones_bf[0:1], ones_bf[0:1], chi[0:1], clo[0:1],
                          chi[0:1], chi[1:2], clo[1:2], chi[1:2],
                          shi[0:1], slo[0:1]]
                    kg = [shi[0:1], slo[0:1], chi[0:1], chi[0:1], clo[0:1],
                          chi[1:2], chi[1:2], clo[1:2],
                          ones_bf[0:1], ones_bf[0:1]]
                    qr = persist.tile([96 + KR, P], F16, tag=f"qrows{s}")
                    kr = persist.tile([96 + KR, P], F16, tag=f"krows{s}")
                    for rg in (0, 32, 64, 96):
                        for r in range(KR):
                            nc.sync.dma_start(out=qr[rg + r:rg + r + 1, :],
                                              in_=qg[r])
                            nc.sync.dma_start(out=kr[rg + r:rg + r + 1, :],
                                              in_=kg[r])
                    qrows.append(qr)
                    krows.append(kr)

            # ---------------- main: 4 generations ----------------
            with tc.tile_pool(name="mm_psum", bufs=4, space="PSUM") as mpsum:
                cp_pool = ctx.enter_context(tc.tile_pool(name="cp", bufs=3))

                def emit_itile(g, t, rA, rB):
                    qi, ki = gen_sets[g]
                    lhsT, rhs = qrows[qi], krows[ki]
                    # PSUM chunks: buffers 0,1 = N1,N2 (ScalarE-drained),
                    # 2,3 = D1,D2 (DVE reads PSUM directly). Alloc order maps
                    # chunks onto pool buffers so PE(t+1) reuses banks freed
                    # earliest (N* by ScalarE, D* by the fold).
                    chunks = [mpsum.tile([128, CH], F32, tag="ps", name="ps")
                              for _ in range(4)]
                    order = (2, 3, 0, 1)  # key-chunk index of N1,N2,D1,D2
                    for buf, kc in enumerate(order):
                        ps = chunks[buf]
                        for h in range(2):
                            k = 2 * kc + h        # 512-col matmul index 0..7
                            rg = 32 * (k % 4)
                            j0 = k * 512
                            nc.tensor.matmul(
                                ps[:, h * 512:(h + 1) * 512],
                                lhsT[rg:rg + KR, t * 128:(t + 1) * 128],
                                rhs[rg:rg + KR, j0:j0 + 512],
                                start=True, stop=True,
                                tile_position=(rg, 0),
                            )
                    n1, n2, d1, d2 = chunks
                    cp1 = cp_pool.tile([128, CH], F32, tag="cp1")
                    cp2 = cp_pool.tile([128, CH], F32, tag="cp2")
                    nc.scalar.copy(cp1[:], n1[:])
                    nc.scalar.copy(cp2[:], n2[:])
                    nc.vector._custom_dve(
                        maxmax, out=trash[:],
                        in0=d1[:], in1=cp1[:],
                        s0=NEG_BIG, accum_out=rA[:, t:t + 1],
                    )
                    nc.vector._custom_dve(
                        maxmax, out=trash[:],
                        in0=d2[:], in1=cp2[:],
                        s0=NEG_BIG, accum_out=rB[:, t:t + 1],
                    )

                def finish_gen(g, rA, rB):
                    # PSUM held -d^2/2; min_j d^2 = -2 * max
                    rm = temps.tile([128, IT], F32, tag="rm")
                    nc.vector.tensor_tensor(rm[:], rA[:], rB[:], op=ALU.max)
                    minsq = temps.tile([128, IT], F32, tag="minsq")
                    nc.vector.tensor_scalar_mul(minsq[:], rm[:], -2.0)
                    nc.sync.dma_start(out=o[g], in_=minsq[:])

                def body(_iv=None):
                    # interleave two independent generations per round so the
                    # scheduler always has a second dependency chain to hide
                    # PSUM-buffer turnaround bubbles
                    for ga, gb in ((0, 2), (1, 3)):
                        accs = {}
                        for g in (ga, gb):
                            rA = persist.tile([128, IT], F32, tag=f"rA{g}",
                                              name="rA")
                            rB = persist.tile([128, IT], F32, tag=f"rB{g}",
                                              name="rB")
                            accs[g] = (rA, rB)
                        for t in range(IT):
                            for g in (ga, gb):
                                emit_itile(g, t, *accs[g])
                        for g in (ga, gb):
                            finish_gen(g, *accs[g])

                if repeat == 1:
                    body()
                else:
                    with tc.For_i(0, repeat, 1):
                        body()

    nc.compile()
    return nc


_CACHE = {}
LAST_RESULTS = None


def _get_program(repeat: int = 1):
    key = ("nc", repeat)
    if key not in _CACHE:
        _CACHE[key] = _build_program(repeat)
    return _CACHE[key]


def kernel(point_set_1: np.ndarray, point_set_2: np.ndarray,
           _trace: bool = False, _repeat: int = 1) -> np.ndarray:
    global LAST_RESULTS
    point_set_1 = np.ascontiguousarray(point_set_1, dtype=np.float32)
    point_set_2 = np.ascontiguousarray(point_set_2, dtype=np.float32)
    assert point_set_1.shape == (N_OBJ, P, 2) and point_set_2.shape == (N_OBJ, P, 2)

    nc = _get_program(_repeat)
    in_maps = []
    for c in range(N_CORES):
        sl = slice(c * OBJ_PER_CORE, (c + 1) * OBJ_PER_CORE)
        in_maps.append({
            "pts1": np.ascontiguousarray(point_set_1[sl]),
            "pts2": np.ascontiguousarray(point_set_2[sl]),
        })
    res = run_bass_kernel_spmd(
        nc, in_maps, core_ids=list(range(N_CORES)), trace=_trace,
    )
    LAST_RESULTS = res

    # host finish: minima -> sqrt -> means -> mask -> final mean
    costs = np.zeros(N_OBJ, dtype=np.float64)
    for c in range(N_CORES):
        minsq = res.results[c]["minsq"]          # [4, 128, IT]
        for obj in range(OBJ_PER_CORE):
            n = c * OBJ_PER_CORE + obj
            d_sum = 0.0
            for direction in range(2):
                g = 2 * obj + direction
                ms = minsq[g].T.reshape(P)       # i = t*128 + m
                d = np.sqrt(np.maximum(ms.astype(np.float64), EPS))
                d_sum += d.mean()
            costs[n] = 0.5 * d_sum
    mask = (point_set_2.reshape(N_OBJ, -1).sum(axis=1, dtype=np.float32) >= 0)
    loss = (costs * mask).sum() / N_OBJ
    return np.asarray(loss, dtype=np.float32)


# revision 24
# speedup vs baseline: 1.1709x; 1.1709x over previous
"""Chamfer 2D loss kernel for Trainium2 (8 NeuronCores, SPMD).

Problem: N=16 objects, two point sets [16, 4096, 2] fp32 each.
Per object: C[i,j] = sqrt(clip(|x_i|^2 + |y_j|^2 - 2 x_i.y_j, 1e-12));
loss = mean_n mask_n * 0.5 * (mean_i min_j C + mean_j min_i C).

Sharding: data-parallel over objects, 2 objects per core; per core 4
"generations" (2 objects x 2 directions), each computing the 4096
per-query min squared distances for query set Q vs key set K.

Per generation, per i-tile of 128 queries, ALL 4096 keys go through the
PE as a K=10 fp16 matmul that computes -d^2/2 = q.k - |k|^2/2 - |q|^2/2
directly in PSUM.  Full fp32 inputs are split hi/lo into fp16 pairs
(k = kh + kl), keeping ~22 mantissa bits through the product terms:
  r0 (1,    mh_k) r1 (1,    ml_k)      m = -|k|^2/2 split
  r2 (q0h, k0h) r3 (q0l, k0h) r4 (q0h, k0l)
  r5 (q1h, k1h) r6 (q1l, k1h) r7 (q1h, k1l)
  r8 (sqh_q, 1) r9 (sql_q, 1)          sq = -|q|^2/2 split
fp16 matmuls stream 1 column/cycle (vs 4 for fp32), so the PE is far off
the critical path.  Four 1024-key PSUM chunks per i-tile (2 banks each,
4-buf pool = all 8 banks): chunks N1,N2 are drained PSUM->SBUF by
ScalarE; chunks D1,D2 are read directly from PSUM by the custom DVE
max-fold (port0=PSUM, port1=SBUF, 2 keys/cycle), accumulating row maxima
of -d^2/2 into per-i-tile columns.  min_j d^2 = -2 * max.
min over j of sqrt == sqrt of min over j (monotonic), so only the 4096
per-query minima need sqrt, done on host with the means and the mask.
"""

import contextlib

import numpy as np

import concourse.bacc as bacc
import concourse.tile as tile
from concourse import mybir
import concourse.dve_ops as dve_ops
from concourse.dve_ops import DveOp
from concourse.dve_spec import (
    Spec, Src0, Src1, C0, maxx, lower, _has_src1,
)
from concourse.dve_uop import DveOpSpec
from concourse.bass_utils import run_bass_kernel_spmd

F32 = mybir.dt.float32
F16 = mybir.dt.float16
ALU = mybir.AluOpType

N_CORES = 8
N_OBJ = 16
P = 4096          # points per set
OBJ_PER_CORE = N_OBJ // N_CORES      # 2
N_GENS = 2 * OBJ_PER_CORE            # 4 generations per core
IT = P // 128                        # 32 i-tiles of 128 queries
CH = 1024                            # keys per PSUM chunk (2 banks)
KR = 10                              # matmul contraction rows per group
EPS = 1e-12
NEG_BIG = -3.0e38


def _maxmax_ref(in0, in1, s0, s1, imm2):
    b = np.maximum(in0, in1).astype(np.float32)
    acc = np.maximum(
        np.asarray(s0, np.float32),
        b.reshape(b.shape[0], -1).max(axis=-1, keepdims=True),
    ).astype(np.float32)
    return b, acc


_REGISTERED = {}


def _register_op(name: str, spec: Spec) -> DveOp:
    """Register a custom DVE op at runtime. Tables are generated per-NEFF,
    so this needs no firmware support."""
    if name in _REGISTERED:
        return _REGISTERED[name]
    for op in dve_ops.OPS:
        if op.name == name:
            _REGISTERED[name] = op
            return op
    row = max(dve_ops._SUB_OPCODE_FOR_NAME.values()) + 1
    assert row < 0x20, "no free custom-DVE opcode row"
    dve_ops._SUB_OPCODE_FOR_NAME[name] = row
    shas = {
        ver: DveOpSpec(
            name=name, opcode=row, uops=lower(spec, ver=ver),
            rd1_en=_has_src1(spec),
        ).sha(ver)
        for ver in ("v3", "v4")
    }
    op = DveOp(name, spec, subdim=False, uops_sha=shas)
    dve_ops.OPS.append(op)
    dve_ops.CUSTOM_DVE_SPECS[name] = spec
    _REGISTERED[name] = op
    return op


def _get_ops():
    maxmax = _register_op(
        "TT_MAX_MAX_REDUCE_ANT",
        Spec(body=maxx(Src0, Src1), accum=maxx, accum_init=C0,
             reference=_maxmax_ref),
    )
    return maxmax


def _build_program(repeat: int = 1):
    """Build + compile the per-core Bass program. `repeat` re-runs the main
    compute loop inside a hardware For_i for timing (results unchanged)."""
    maxmax = _get_ops()

    nc = bacc.Bacc("TRN2", target_bir_lowering=False, debug=False)
    pts1 = nc.dram_tensor("pts1", [OBJ_PER_CORE, P, 2], F32, kind="ExternalInput")
    pts2 = nc.dram_tensor("pts2", [OBJ_PER_CORE, P, 2], F32, kind="ExternalInput")
    out = nc.dram_tensor("minsq", [N_GENS, 128, IT], F32, kind="ExternalOutput")

    ones_row = nc.inline_tensor(np.ones((1, P), dtype=np.float32), name="ones_row")
    scr = [nc.dram_tensor(f"scr{s}", [1, P], F32, kind="Internal") for s in range(4)]

    p1 = pts1.ap()
    p2 = pts2.ap()
    o = out.ap()

    # point sets in per-core order: s = 2*obj + (0: set1, 1: set2)
    set_aps = [p1[0], p2[0], p1[1], p2[1]]
    # generations: (query set idx, key set idx)
    gen_sets = [(0, 1), (1, 0), (2, 3), (3, 2)]

    with tile.TileContext(nc) as tc:
        with contextlib.ExitStack() as ctx:
            persist = ctx.enter_context(tc.tile_pool(name="persist", bufs=1))
            temps = ctx.enter_context(tc.tile_pool(name="temps", bufs=2))

            # discarded elementwise outputs of the custom DVE fold ops,
            # rotated through a pool so no fold is WAW-coupled to the
            # previous i-tile's folds
            trash_pool = ctx.enter_context(tc.tile_pool(name="trash", bufs=2))
            qrows, krows = [], []
            with tc.tile_pool(name="prep_psum", bufs=2, space="PSUM") as ppsum, \
                 tc.tile_pool(name="prep", bufs=1) as prep:
                from concourse.masks import make_identity
                ident = prep.tile([128, 128], F32, tag="ident")
                make_identity(nc, ident[:])
                ones_bf = prep.tile([1, P], F16, tag="ones_bf")
                ones_sb = prep.tile([1, P], F32, tag="ones_sb")
                nc.sync.dma_start(out=ones_sb[:], in_=ones_row.ap()[:])
                nc.vector.tensor_copy(out=ones_bf[:], in_=ones_sb[:])

                def hi_lo(src_ap, n_part, tag):
                    """Split fp32 rows [n, P] into fp16 hi + lo tiles."""
                    hi = prep.tile([n_part, P], F16, tag=f"{tag}_hi", name="hi")
                    lo = prep.tile([n_part, P], F16, tag=f"{tag}_lo", name="lo")
                    hi32 = prep.tile([n_part, P], F32, tag="hl32", name="hi32")
                    nc.vector.tensor_copy(out=hi[:], in_=src_ap)
                    nc.vector.tensor_copy(out=hi32[:], in_=hi[:])
                    nc.vector.tensor_tensor(hi32[:], src_ap, hi32[:],
                                            op=ALU.subtract)
                    nc.vector.tensor_copy(out=lo[:], in_=hi32[:])
                    return hi, lo

                for s in range(4):
                    pset = set_aps[s]          # [P, 2] dram AP
                    rows2 = pset.rearrange("p d -> d p")              # [2, P]
                    cols = pset.rearrange("(c p) d -> p c d", p=128)  # [128, IT, 2]

                    # coordinate rows (fp32) -> fp16 hi/lo [2, P]
                    crows = prep.tile([2, P], F32, tag="crows")
                    nc.sync.dma_start(out=crows[:], in_=rows2)
                    chi, clo = hi_lo(crows[:], 2, "c")

                    # -|p|^2/2 row: build via column-wise square/sum, PE
                    # transpose, dram bounce -> [1, P] fp32 -> fp16 hi/lo
                    c0 = prep.tile([128, IT], F32, tag="qc0")
                    c1 = prep.tile([128, IT], F32, tag="qc1")
                    nc.sync.dma_start(out=c0[:], in_=cols[:, :, 0])
                    nc.sync.dma_start(out=c1[:], in_=cols[:, :, 1])
                    m0 = prep.tile([128, IT], F32, tag="m0")
                    sqc = prep.tile([128, IT], F32, tag="sqc")
                    nc.vector.tensor_tensor(m0[:], c0[:], c0[:], op=ALU.mult)
                    nc.vector.tensor_tensor(sqc[:], c1[:], c1[:], op=ALU.mult)
                    nc.vector.tensor_tensor(sqc[:], sqc[:], m0[:], op=ALU.add)
                    nsq = prep.tile([128, IT], F32, tag="nsq")
                    nc.vector.tensor_scalar_mul(nsq[:], sqc[:], -0.5)
                    pT = ppsum.tile([IT, 128], F32, tag="pT")
                    nc.tensor.transpose(pT[:], nsq[:], ident[:])
                    pTs = prep.tile([IT, 128], F32, tag="pTs")
                    nc.vector.tensor_copy(out=pTs[:], in_=pT[:])
                    nc.sync.dma_start(
                        out=scr[s].ap()[0].rearrange("(c p) -> c p", p=128),
                        in_=pTs[:],
                    )
                    srow = prep.tile([1, P], F32, tag="srow")
                    nc.sync.dma_start(out=srow[:], in_=scr[s].ap()[:])
                    shi, slo = hi_lo(srow[:], 1, "s")

                    # assemble the replicated row-group tiles (see module
                    # docstring for the 10-row pairing)
                    qg = [ones_bf[0:1], ones_bf[0:1], chi[0:1], clo[0:1],
                          chi[0:1], chi[1:2], clo[1:2], chi[1:2],
                          shi[0:1], slo[0:1]]
                    kg = [shi[0:1], slo[0:1], chi[0:1], chi[0:1], clo[0:1],
                          chi[1:2], chi[1:2], clo[1:2],
                          ones_bf[0:1], ones_bf[0:1]]
                    qr = persist.tile([96 + KR, P], F16, tag=f"qrows{s}")
                    kr = persist.tile([96 + KR, P], F16, tag=f"krows{s}")
                    for rg in (0, 32, 64, 96):
                        for r in range(KR):
                            nc.sync.dma_start(out=qr[rg + r:rg + r + 1, :],
                                              in_=qg[r])
                            nc.sync.dma_start(out=kr[rg + r:rg + r + 1, :],
                                              in_=kg[r])
                    qrows.append(qr)
                    krows.append(kr)

            # ---------------- main: 4 generations ----------------
            with tc.tile_pool(name="mm_psum", bufs=4, space="PSUM") as mpsum:
                cp_pool = ctx.enter_context(tc.tile_pool(name="cp", bufs=4))

                def emit_itile(g, t, rA, rB):
                    qi, ki = gen_sets[g]
                    lhsT, rhs = qrows[qi], krows[ki]
                    # PSUM chunks: buffers 0,1 = N1,N2 (ScalarE-drained),
                    # 2,3 = D1,D2 (DVE reads PSUM directly). Alloc order maps
                    # chunks onto pool buffers so PE(t+1) reuses banks freed
                    # earliest (N* by ScalarE, D* by the fold).
                    chunks = [mpsum.tile([128, CH], F32, tag="ps", name="ps")
                              for _ in range(4)]
                    order = (2, 3, 0, 1)  # key-chunk index of N1,N2,D1,D2
                    for buf, kc in enumerate(order):
                        ps = chunks[buf]
                        for h in range(2):
                            k = 2 * kc + h        # 512-col matmul index 0..7
                            rg = 32 * (k % 4)
                            j0 = k * 512
                            nc.tensor.matmul(
                                ps[:, h * 512:(h + 1) * 512],
                                lhsT[rg:rg + KR, t * 128:(t + 1) * 128],
                                rhs[rg:rg + KR, j0:j0 + 512],
                                start=True, stop=True,
                                tile_position=(rg, 0),
                            )
                    n1, n2, d1, d2 = chunks
                    cp1 = cp_pool.tile([128, CH], F32, tag="cp1")
                    cp2 = cp_pool.tile([128, CH], F32, tag="cp2")
                    nc.scalar.copy(cp1[:], n1[:])
                    nc.scalar.copy(cp2[:], n2[:])
                    tr1 = trash_pool.tile([128, CH], F32, tag="t1", name="tr")
                    tr2 = trash_pool.tile([128, CH], F32, tag="t2", name="tr")
                    nc.vector._custom_dve(
                        maxmax, out=tr1[:],
                        in0=d1[:], in1=cp1[:],
                        s0=NEG_BIG, accum_out=rA[:, t:t + 1],
                    )
                    nc.vector._custom_dve(
                        maxmax, out=tr2[:],
                        in0=d2[:], in1=cp2[:],
                        s0=NEG_BIG, accum_out=rB[:, t:t + 1],
                    )

                def finish_gen(g, rA, rB):
                    # PSUM held -d^2/2; min_j d^2 = -2 * max
                    rm = temps.tile([128, IT], F32, tag="rm")
                    nc.vector.tensor_tensor(rm[:], rA[:], rB[:], op=ALU.max)
                    minsq = temps.tile([128, IT], F32, tag="minsq")
                    nc.vector.tensor_scalar_mul(minsq[:], rm[:], -2.0)
                    nc.sync.dma_start(out=o[g], in_=minsq[:])

                def body(_iv=None):
                    for g in range(N_GENS):
                        rA = persist.tile([128, IT], F32, tag=f"rA{g}",
                                          name="rA")
                        rB = persist.tile([128, IT], F32, tag=f"rB{g}",
                                          name="rB")
                        for t in range(IT):
                            emit_itile(g, t, rA, rB)
                        finish_gen(g, rA, rB)

                if repeat == 1:
                    body()
                else:
                    with tc.For_i(0, repeat, 1):
                        body()

    nc.compile()
    return nc


_CACHE = {}
LAST_RESULTS = None


def _get_program(repeat: int = 1):
    key = ("nc", repeat)
    if key not in _CACHE:
        _CACHE[key] = _build_program(repeat)
    return _CACHE[key]


def kernel(point_set_1: np.ndarray, point_set_2: np.ndarray,
           _trace: bool = False, _repeat: int = 1) -> np.ndarray:
    global LAST_RESULTS
    point_set_1 = np.ascontiguousarray(point_set_1, dtype=np.float32)
    point_set_2 = np.ascontiguousarray(point_set_2, dtype=np.float32)
    assert point_set_1.shape == (N_OBJ, P, 2) and point_set_2.shape == (N_OBJ, P, 2)

    nc = _get_program(_repeat)
    in_maps = []
    for c in range(N_CORES):
        sl = slice(c * OBJ_PER_CORE, (c + 1) * OBJ_PER_CORE)
        in_maps.append({
            "pts1": np.ascontiguousarray(point_set_1[sl]),
            "pts2": np.ascontiguousarray(point_set_2[sl]),
        })
    res = run_bass_kernel_spmd(
        nc, in_maps, core_ids=list(range(N_CORES)), trace=_trace,
    )
    LAST_RESULTS = res

    # host finish: minima -> sqrt -> means -> mask -> final mean
    costs = np.zeros(N_OBJ, dtype=np.float64)
    for c in range(N_CORES):
        minsq = res.results[c]["minsq"]          # [4, 128, IT]
        for obj in range(OBJ_PER_CORE):
            n = c * OBJ_PER_CORE + obj
            d_sum = 0.0
            for direction in range(2):
                g = 2 * obj + direction
                ms = minsq[g].T.reshape(P)       # i = t*128 + m
                d = np.sqrt(np.maximum(ms.astype(np.float64), EPS))
                d_sum += d.mean()
            costs[n] = 0.5 * d_sum
    mask = (point_set_2.reshape(N_OBJ, -1).sum(axis=1, dtype=np.float32) >= 0)
    loss = (costs * mask).sum() / N_OBJ
    return np.asarray(loss, dtype=np.float32)


# revision 26
# speedup vs baseline: 1.2743x; 1.0883x over previous
"""Chamfer 2D loss kernel for Trainium2 (8 NeuronCores, SPMD).

Problem: N=16 objects, two point sets [16, 4096, 2] fp32 each.
Per object: C[i,j] = sqrt(clip(|x_i|^2 + |y_j|^2 - 2 x_i.y_j, 1e-12));
loss = mean_n mask_n * 0.5 * (mean_i min_j C + mean_j min_i C).

Sharding: data-parallel over objects, 2 objects per core; per core 4
"generations" (2 objects x 2 directions), each computing the 4096
per-query min squared distances for query set Q vs key set K.

Per generation, per i-tile of 128 queries, ALL 4096 keys go through the
PE as a K=10 fp16 matmul that computes -d^2/2 = q.k - |k|^2/2 - |q|^2/2
directly in PSUM.  Full fp32 inputs are split hi/lo into fp16 pairs
(k = kh + kl), keeping ~22 mantissa bits through the product terms:
  r0 (1,    mh_k) r1 (1,    ml_k)      m = -|k|^2/2 split
  r2 (q0h, k0h) r3 (q0l, k0h) r4 (q0h, k0l)
  r5 (q1h, k1h) r6 (q1l, k1h) r7 (q1h, k1l)
  r8 (sqh_q, 1) r9 (sql_q, 1)          sq = -|q|^2/2 split
fp16 matmuls stream 1 column/cycle (vs 4 for fp32), so the PE is far off
the critical path.  Four 1024-key PSUM chunks per i-tile (2 banks each,
4-buf pool = all 8 banks): chunks N1,N2 are drained PSUM->SBUF by
ScalarE; chunks D1,D2 are read directly from PSUM by the custom DVE
max-fold (port0=PSUM, port1=SBUF, 2 keys/cycle), accumulating row maxima
of -d^2/2 into per-i-tile columns.  min_j d^2 = -2 * max.
min over j of sqrt == sqrt of min over j (monotonic), so only the 4096
per-query minima need sqrt, done on host with the means and the mask.
"""

import contextlib

import numpy as np

import concourse.bacc as bacc
import concourse.tile as tile
from concourse import mybir
import concourse.dve_ops as dve_ops
from concourse.dve_ops import DveOp
from concourse.dve_spec import (
    Spec, Src0, Src1, C0, maxx, lower, _has_src1,
)
from concourse.dve_uop import DveOpSpec
from concourse.bass_utils import run_bass_kernel_spmd

F32 = mybir.dt.float32
F16 = mybir.dt.float16
ALU = mybir.AluOpType

N_CORES = 8
N_OBJ = 16
P = 4096          # points per set
OBJ_PER_CORE = N_OBJ // N_CORES      # 2
N_GENS = 2 * OBJ_PER_CORE            # 4 generations per core
IT = P // 128                        # 32 i-tiles of 128 queries
CH = 1024                            # keys per PSUM chunk (2 banks)
KR = 10                              # matmul contraction rows per group
EPS = 1e-12
NEG_BIG = -3.0e38


def _maxmax_ref(in0, in1, s0, s1, imm2):
    b = np.maximum(in0, in1).astype(np.float32)
    acc = np.maximum(
        np.asarray(s0, np.float32),
        b.reshape(b.shape[0], -1).max(axis=-1, keepdims=True),
    ).astype(np.float32)
    return b, acc


_REGISTERED = {}


def _register_op(name: str, spec: Spec) -> DveOp:
    """Register a custom DVE op at runtime. Tables are generated per-NEFF,
    so this needs no firmware support."""
    if name in _REGISTERED:
        return _REGISTERED[name]
    for op in dve_ops.OPS:
        if op.name == name:
            _REGISTERED[name] = op
            return op
    row = max(dve_ops._SUB_OPCODE_FOR_NAME.values()) + 1
    assert row < 0x20, "no free custom-DVE opcode row"
    dve_ops._SUB_OPCODE_FOR_NAME[name] = row
    shas = {
        ver: DveOpSpec(
            name=name, opcode=row, uops=lower(spec, ver=ver),
            rd1_en=_has_src1(spec),
        ).sha(ver)
        for ver in ("v3", "v4")
    }
    op = DveOp(name, spec, subdim=False, uops_sha=shas)
    dve_ops.OPS.append(op)
    dve_ops.CUSTOM_DVE_SPECS[name] = spec
    _REGISTERED[name] = op
    return op


def _get_ops():
    maxmax = _register_op(
        "TT_MAX_MAX_REDUCE_ANT",
        Spec(body=maxx(Src0, Src1), accum=maxx, accum_init=C0,
             reference=_maxmax_ref),
    )
    return maxmax


def _build_program(repeat: int = 1):
    """Build + compile the per-core Bass program. `repeat` re-runs the main
    compute loop inside a hardware For_i for timing (results unchanged)."""
    maxmax = _get_ops()

    nc = bacc.Bacc("TRN2", target_bir_lowering=False, debug=False)
    pts1 = nc.dram_tensor("pts1", [OBJ_PER_CORE, P, 2], F32, kind="ExternalInput")
    pts2 = nc.dram_tensor("pts2", [OBJ_PER_CORE, P, 2], F32, kind="ExternalInput")
    out = nc.dram_tensor("minsq", [N_GENS, 128, IT], F32, kind="ExternalOutput")

    ones_row = nc.inline_tensor(np.ones((1, P), dtype=np.float32), name="ones_row")
    scr = [nc.dram_tensor(f"scr{s}", [1, P], F32, kind="Internal") for s in range(4)]

    p1 = pts1.ap()
    p2 = pts2.ap()
    o = out.ap()

    # point sets in per-core order: s = 2*obj + (0: set1, 1: set2)
    set_aps = [p1[0], p2[0], p1[1], p2[1]]
    # generations: (query set idx, key set idx)
    gen_sets = [(0, 1), (1, 0), (2, 3), (3, 2)]

    with tile.TileContext(nc) as tc:
        with contextlib.ExitStack() as ctx:
            persist = ctx.enter_context(tc.tile_pool(name="persist", bufs=1))
            temps = ctx.enter_context(tc.tile_pool(name="temps", bufs=2))

            # discarded elementwise outputs of the custom DVE fold ops,
            # rotated through a pool so no fold is WAW-coupled to the
            # previous i-tile's folds
            trash_pool = ctx.enter_context(tc.tile_pool(name="trash", bufs=2))
            qrows, krows = [], []
            with tc.tile_pool(name="prep_psum", bufs=2, space="PSUM") as ppsum, \
                 tc.tile_pool(name="prep", bufs=1) as prep:
                from concourse.masks import make_identity
                ident = prep.tile([128, 128], F32, tag="ident")
                make_identity(nc, ident[:])
                ones_bf = prep.tile([1, P], F16, tag="ones_bf")
                ones_sb = prep.tile([1, P], F32, tag="ones_sb")
                nc.sync.dma_start(out=ones_sb[:], in_=ones_row.ap()[:])
                nc.vector.tensor_copy(out=ones_bf[:], in_=ones_sb[:])

                def hi_lo(src_ap, n_part, tag):
                    """Split fp32 rows [n, P] into fp16 hi + lo tiles."""
                    hi = prep.tile([n_part, P], F16, tag=f"{tag}_hi", name="hi")
                    lo = prep.tile([n_part, P], F16, tag=f"{tag}_lo", name="lo")
                    hi32 = prep.tile([n_part, P], F32, tag="hl32", name="hi32")
                    nc.vector.tensor_copy(out=hi[:], in_=src_ap)
                    nc.vector.tensor_copy(out=hi32[:], in_=hi[:])
                    nc.vector.tensor_tensor(hi32[:], src_ap, hi32[:],
                                            op=ALU.subtract)
                    nc.vector.tensor_copy(out=lo[:], in_=hi32[:])
                    return hi, lo

                for s in range(4):
                    pset = set_aps[s]          # [P, 2] dram AP
                    rows2 = pset.rearrange("p d -> d p")              # [2, P]
                    cols = pset.rearrange("(c p) d -> p c d", p=128)  # [128, IT, 2]

                    # coordinate rows (fp32) -> fp16 hi/lo [2, P]
                    crows = prep.tile([2, P], F32, tag="crows")
                    nc.sync.dma_start(out=crows[:], in_=rows2)
                    chi, clo = hi_lo(crows[:], 2, "c")

                    # -|p|^2/2 row: build via column-wise square/sum, PE
                    # transpose, dram bounce -> [1, P] fp32 -> fp16 hi/lo
                    c0 = prep.tile([128, IT], F32, tag="qc0")
                    c1 = prep.tile([128, IT], F32, tag="qc1")
                    nc.sync.dma_start(out=c0[:], in_=cols[:, :, 0])
                    nc.sync.dma_start(out=c1[:], in_=cols[:, :, 1])
                    m0 = prep.tile([128, IT], F32, tag="m0")
                    sqc = prep.tile([128, IT], F32, tag="sqc")
                    nc.vector.tensor_tensor(m0[:], c0[:], c0[:], op=ALU.mult)
                    nc.vector.tensor_tensor(sqc[:], c1[:], c1[:], op=ALU.mult)
                    nc.vector.tensor_tensor(sqc[:], sqc[:], m0[:], op=ALU.add)
                    nsq = prep.tile([128, IT], F32, tag="nsq")
                    nc.vector.tensor_scalar_mul(nsq[:], sqc[:], -0.5)
                    pT = ppsum.tile([IT, 128], F32, tag="pT")
                    nc.tensor.transpose(pT[:], nsq[:], ident[:])
                    pTs = prep.tile([IT, 128], F32, tag="pTs")
                    nc.vector.tensor_copy(out=pTs[:], in_=pT[:])
                    nc.sync.dma_start(
                        out=scr[s].ap()[0].rearrange("(c p) -> c p", p=128),
                        in_=pTs[:],
                    )
                    srow = prep.tile([1, P], F32, tag="srow")
                    nc.sync.dma_start(out=srow[:], in_=scr[s].ap()[:])
                    shi, slo = hi_lo(srow[:], 1, "s")

                    # assemble the replicated row-group tiles (see module
                    # docstring for the 10-row pairing)
                    qg = [ones_bf[0:1], ones_bf[0:1], chi[0:1], clo[0:1],
                          chi[0:1], chi[1:2], clo[1:2], chi[1:2],
                          shi[0:1], slo[0:1]]
                    kg = [shi[0:1], slo[0:1], chi[0:1], chi[0:1], clo[0:1],
                          chi[1:2], chi[1:2], clo[1:2],
                          ones_bf[0:1], ones_bf[0:1]]
                    qr = persist.tile([96 + KR, P], F16, tag=f"qrows{s}")
                    kr = persist.tile([96 + KR, P], F16, tag=f"krows{s}")
                    for rg in (0, 32, 64, 96):
                        for r in range(KR):
                            nc.sync.dma_start(out=qr[rg + r:rg + r + 1, :],
                                              in_=qg[r])
                            nc.sync.dma_start(out=kr[rg + r:rg + r + 1, :],
                                              in_=kg[r])
                    qrows.append(qr)
                    krows.append(kr)

            # ---------------- main: 4 generations ----------------
            with tc.tile_pool(name="mm_psum", bufs=4, space="PSUM") as mpsum:
                cp_pool = ctx.enter_context(tc.tile_pool(name="cp", bufs=4))

                def emit_itile(g, t, rA, rB):
                    qi, ki = gen_sets[g]
                    lhsT, rhs = qrows[qi], krows[ki]
                    # PSUM chunks: buffers 0,1 = N1,N2 (ScalarE-drained),
                    # 2,3 = D1,D2 (DVE reads PSUM directly). Alloc order maps
                    # chunks onto pool buffers so PE(t+1) reuses banks freed
                    # earliest (N* by ScalarE, D* by the fold).
                    chunks = [mpsum.tile([128, CH], F32, tag="ps", name="ps")
                              for _ in range(4)]
                    order = (2, 0, 3, 1)  # key-chunk index of N1,D1,N2,D2
                    for buf, kc in enumerate(order):
                        ps = chunks[buf]
                        for h in range(2):
                            k = 2 * kc + h        # 512-col matmul index 0..7
                            rg = 32 * (k % 4)
                            j0 = k * 512
                            nc.tensor.matmul(
                                ps[:, h * 512:(h + 1) * 512],
                                lhsT[rg:rg + KR, t * 128:(t + 1) * 128],
                                rhs[rg:rg + KR, j0:j0 + 512],
                                start=True, stop=True,
                                tile_position=(rg, 0),
                            )
                    n1, d1, n2, d2 = chunks
                    cp1 = cp_pool.tile([128, CH], F32, tag="cp1")
                    cp2 = cp_pool.tile([128, CH], F32, tag="cp2")
                    nc.scalar.copy(cp1[:], n1[:])
                    nc.scalar.copy(cp2[:], n2[:])
                    tr1 = trash_pool.tile([128, CH], F32, tag="t1", name="tr")
                    tr2 = trash_pool.tile([128, CH], F32, tag="t2", name="tr")
                    nc.vector._custom_dve(
                        maxmax, out=tr1[:],
                        in0=d1[:], in1=cp1[:],
                        s0=NEG_BIG, accum_out=rA[:, t:t + 1],
                    )
                    nc.vector._custom_dve(
                        maxmax, out=tr2[:],
                        in0=d2[:], in1=cp2[:],
                        s0=NEG_BIG, accum_out=rB[:, t:t + 1],
                    )

                def finish_gen(g, rA, rB):
                    # PSUM held -d^2/2; min_j d^2 = -2 * max
                    rm = temps.tile([128, IT], F32, tag="rm")
                    nc.vector.tensor_tensor(rm[:], rA[:], rB[:], op=ALU.max)
                    minsq = temps.tile([128, IT], F32, tag="minsq")
                    nc.vector.tensor_scalar_mul(minsq[:], rm[:], -2.0)
                    nc.sync.dma_start(out=o[g], in_=minsq[:])

                def body(_iv=None):
                    for g in range(N_GENS):
                        rA = persist.tile([128, IT], F32, tag=f"rA{g}",
                                          name="rA")
                        rB = persist.tile([128, IT], F32, tag=f"rB{g}",
                                          name="rB")
                        for t in range(IT):
                            emit_itile(g, t, rA, rB)
                        finish_gen(g, rA, rB)

                if repeat == 1:
                    body()
                else:
                    with tc.For_i(0, repeat, 1):
                        body()

    nc.compile()
    return nc


_CACHE = {}
LAST_RESULTS = None


def _get_program(repeat: int = 1):
    key = ("nc", repeat)
    if key not in _CACHE:
        _CACHE[key] = _build_program(repeat)
    return _CACHE[key]


def kernel(point_set_1: np.ndarray, point_set_2: np.ndarray,
           _trace: bool = False, _repeat: int = 1) -> np.ndarray:
    global LAST_RESULTS
    point_set_1 = np.ascontiguousarray(point_set_1, dtype=np.float32)
    point_set_2 = np.ascontiguousarray(point_set_2, dtype=np.float32)
    assert point_set_1.shape == (N_OBJ, P, 2) and point_set_2.shape == (N_OBJ, P, 2)

    nc = _get_program(_repeat)
    in_maps = []
    for c in range(N_CORES):
        sl = slice(c * OBJ_PER_CORE, (c + 1) * OBJ_PER_CORE)
        in_maps.append({
            "pts1": np.ascontiguousarray(point_set_1[sl]),
            "pts2": np.ascontiguousarray(point_set_2[sl]),
        })
    res = run_bass_kernel_spmd(
        nc, in_maps, core_ids=list(range(N_CORES)), trace=_trace,
    )
    LAST_RESULTS = res

    # host finish: minima -> sqrt -> means -> mask -> final mean
    costs = np.zeros(N_OBJ, dtype=np.float64)
    for c in range(N_CORES):
        minsq = res.results[c]["minsq"]          # [4, 128, IT]
        for obj in range(OBJ_PER_CORE):
            n = c * OBJ_PER_CORE + obj
            d_sum = 0.0
            for direction in range(2):
                g = 2 * obj + direction
                ms = minsq[g].T.reshape(P)       # i = t*128 + m
                d = np.sqrt(np.maximum(ms.astype(np.float64), EPS))
                d_sum += d.mean()
            costs[n] = 0.5 * d_sum
    mask = (point_set_2.reshape(N_OBJ, -1).sum(axis=1, dtype=np.float32) >= 0)
    loss = (costs * mask).sum() / N_OBJ
    return np.asarray(loss, dtype=np.float32)
